# revision 12
# baseline (speedup 1.0000x reference)
"""Trainium2 Bass kernel for nn_Attention_43946105373274.

Causal multi-head attention with rotary embeddings applied to q, k and v.
B=2, N=2048, DIM=1024, H=16, DH=64, f32.

Sharding: 8 cores = (2 batches) x (4 head-groups of 4 heads).
Each core computes the qkv projection for its heads (w_qkv column-shard),
full causal attention for its heads, and a partial output projection
(w_out row-shard).  The host sums the 4 partials per batch and adds the
bias (the "all-reduce" of the output projection) — full inputs in, full
output out.

Per-core pipeline:
  0. x^T and freqs^T are transposed on the host (layout prep only), so
     all device DMAs are wide contiguous rows, batched into multi-tile
     transfers and spread across three DMA queues (sync/vector/scalar)
     so the first matmul can start ~5us in.  cos/sin are computed on
     device from freqs^T with a fused round-to-nearest range reduction;
     the pi/2 cosine shift rides the activation bias port.  No PE
     transposes and no fp32 PE instructions anywhere.
  1. QKV projection = bf16 matmuls accumulating in f32 PSUM, producing
     q/k/v in transposed [channels, n] layout (two heads per tile).
     Rotary: rotate_half via one bf16 R-matrix matmul; cos/sin combines
     on DVE in bf16.
  2. Attention is chunk-major with two heads interleaved, and a filler
     queue of independent tensor-engine work (the second half of the
     QKV projection, then output-projection tiles of the previous
     chunk) is popped between attention groups.  The PE therefore never
     idles while the scalar engine computes exp — idle gaps both waste
     cycles and demote the PE clock (2.4 -> 1.2 GHz) for multi-us
     epochs.  S^T = K-block^T q (f32 PSUM), exp on the scalar engine
     (DH^-0.5 folded into the exp scale; max-subtraction is
     mathematically redundant here), causal mask via gpsimd
     affine_select, AV accumulates with a ones-column appended to V so
     softmax row-sums fall out of the same matmul.  Normalization is
     split: the row-sum copy issues right after the last AV, the bf16
     ones-broadcast matmul + reciprocal + scale are deferred one head
     pair to stay off the critical path.
  3. Output-projection tiles ride the filler queue one chunk behind
     attention, so projection matmuls and the f16 output DMAs overlap
     the attention phase instead of serializing after it.
"""

import sys
import numpy as np

if "/opt/trn_rl_repo" not in sys.path:
    sys.path.insert(0, "/opt/trn_rl_repo")

B, N, DIM, H, DH = 2, 2048, 1024, 16, 64
HPC = 4                     # heads per core
NCORES = 8
SCALE = DH ** -0.5
NT = N // 128               # 16 row tiles
KB = DIM // 128             # 8 contraction blocks
CW = 512                    # i-chunk width
NCH = N // CW               # 4 chunks

_CACHE = {}


def _build_program():
    import concourse.bass as bass  # noqa: F401
    import concourse.mybir as mybir
    import concourse.tile as tile
    from concourse import bacc

    F32 = mybir.dt.float32
    F16 = mybir.dt.float16
    BF16 = mybir.dt.bfloat16
    AF = mybir.ActivationFunctionType
    OP = mybir.AluOpType

    nc = bacc.Bacc("TRN2", target_bir_lowering=False, debug=False,
                   num_devices=NCORES)

    xT = nc.dram_tensor("xT", [DIM, N], BF16, kind="ExternalInput")
    wqkv = nc.dram_tensor("wqkv", [DIM, 3 * HPC * DH], BF16, kind="ExternalInput")
    wout = nc.dram_tensor("wout", [HPC * DH, DIM], BF16, kind="ExternalInput")
    freqsT = nc.dram_tensor("freqsT", [DH, N], F32, kind="ExternalInput")
    rmatD = nc.dram_tensor("rmatD", [128, 128], BF16, kind="ExternalInput")
    identB = nc.dram_tensor("identB", [128, 128], BF16, kind="ExternalInput")
    outD = nc.dram_tensor("out", [N, DIM], F16, kind="ExternalOutput")

    MAGIC = 12582912.0          # 1.5 * 2**23: float32 round-to-nearest trick
    TWO_PI = float(2 * np.pi)
    INV_2PI = float(1.0 / TWO_PI)
    HALF_PI = float(np.pi / 2)

    with tile.TileContext(nc) as tc:
        with tc.tile_pool(name="pc", bufs=1) as pc, \
             tc.tile_pool(name="pw", bufs=1) as pw, \
             tc.tile_pool(name="px", bufs=8) as px, \
             tc.tile_pool(name="pqk", bufs=4) as pqk, \
             tc.tile_pool(name="pv", bufs=4) as pv, \
             tc.tile_pool(name="pst", bufs=3) as pst, \
             tc.tile_pool(name="ppt", bufs=6) as ppt, \
             tc.tile_pool(name="poT", bufs=2) as poT, \
             tc.tile_pool(name="pnm", bufs=2) as pnm, \
             tc.tile_pool(name="pout", bufs=3) as pout, \
             tc.tile_pool(name="ptr", bufs=1) as ptr, \
             tc.tile_pool(name="psA", bufs=2, space="PSUM") as psA, \
             tc.tile_pool(name="psB", bufs=3, space="PSUM") as psB:

            # ---------------- phase 0: DMAs on three queues ------------------
            # sync queue: w_qkv (one batched transfer), then x^T cp0 pairs
            wall = pw.tile([128, KB, 3 * HPC * DH], BF16, tag="w")
            nc.sync.dma_start(
                wall[:], wqkv[:].rearrange("(kb p) j -> p kb j", p=128))
            w_sb = [wall[:, kb, :] for kb in range(KB)]

            # scalar queue first: small constants + freqs^T (trig needs them)
            rmat = pc.tile([128, 128], BF16, tag="rmat")
            nc.scalar.dma_start(rmat[:], rmatD[:])
            identb = pc.tile([128, 128], BF16, tag="identb")
            nc.scalar.dma_start(identb[:], identB[:])
            sarg = ptr.tile([128, N], F32, tag="sarg")
            carg = ptr.tile([128, N], F32, tag="carg")
            for half in range(2):
                nc.scalar.dma_start(sarg[half * 64:(half + 1) * 64, :], freqsT[:])
                nc.scalar.dma_start(carg[half * 64:(half + 1) * 64, :], freqsT[:])

            xc = []
            for cp in range(2):
                for pr in range(4):     # pairs of kb
                    t = px.tile([128, 2, 1024], BF16, tag="xT",
                                name=f"xc_{cp}_{pr}")
                    eng = nc.sync if cp == 0 else nc.scalar
                    eng.dma_start(
                        t[:], xT[pr * 256:(pr + 1) * 256,
                                 cp * 1024:(cp + 1) * 1024].rearrange(
                                     "(kb p) n -> p kb n", p=128))
                    xc.append(t)

            def xview(cp, kb):
                return xc[cp * 4 + kb // 2][:, kb % 2, :]

            wot = pw.tile([128, 2, DIM], BF16, tag="wo")
            nc.scalar.dma_start(
                wot[:], wout[:].rearrange("(cb p) d -> p cb d", p=128))
            wo_sb = [wot[:, cb, :] for cb in range(2)]

            ones_f = pc.tile([128, 128], F32, tag="ones_f")
            nc.vector.memset(ones_f[:], 1.0)
            ones_b = pc.tile([1, 128], BF16, tag="ones_b")
            nc.vector.tensor_copy(ones_b[:], ones_f[0:1, :])
            hpi_t = pc.tile([128, 1], F32, tag="hpi_t")
            nc.vector.memset(hpi_t[:], HALF_PI)
            zero_t = pc.tile([128, 1], F32, tag="zero_t")
            nc.vector.memset(zero_t[:], 0.0)

            # cos/sin in bf16, [128, N] (both 64-row halves identical)
            cosT2 = pc.tile([128, N], BF16, tag="cosT2")
            sinT2 = pc.tile([128, N], BF16, tag="sinT2")
            # sin(y) = Sin(y - 2pi*round(y/2pi))
            # cos(y) = Sin((y - 2pi*round(y/2pi + 1/4)) + pi/2)
            for cc in range(NCH):
                sl = slice(cc * CW, (cc + 1) * CW)
                for arg, dst, kofs, bias in (
                        (sarg, sinT2, 0.0, zero_t[:]),
                        (carg, cosT2, 0.25, hpi_t[:])):
                    k = pst.tile([128, CW], F32, tag="trk", bufs=2)
                    nc.vector.tensor_scalar(k[:], arg[:, sl], INV_2PI,
                                            MAGIC + kofs, op0=OP.mult,
                                            op1=OP.add)
                    nc.vector.tensor_scalar_sub(k[:], k[:], MAGIC)
                    xr = pst.tile([128, CW], F32, tag="trx", bufs=2)
                    nc.vector.scalar_tensor_tensor(xr[:], k[:], -TWO_PI,
                                                   arg[:, sl],
                                                   op0=OP.mult, op1=OP.add)
                    nc.scalar.activation(dst[:, sl], xr[:], AF.Sin, bias=bias)

            # persistent tensors
            qT = [pqk.tile([128, N], BF16, tag="qk", name=f"qT{i}") for i in range(2)]
            kT = [pqk.tile([128, N], BF16, tag="qk", name=f"kT{i}") for i in range(2)]
            # V tiles: [128, 65] per (head, row-tile); col 64 = ones
            vt = [pv.tile([128, NT * (DH + 1)], BF16, tag="v", name=f"vt{h}", bufs=4)
                  for h in range(HPC)]
            for h in range(HPC):
                vv = vt[h][:].rearrange("p (t c) -> p t c", c=DH + 1)
                nc.vector.tensor_copy(vv[:, :, DH:DH + 1],
                                      ones_f[:, 0:NT].unsqueeze(2))
            oT = [poT.tile([128, N], BF16, tag="oT", name=f"oT{i}") for i in range(2)]

            # ---------------- phase 1: qkv projection + rotary ---------------
            def emit_qkv_mm(cp, jt, kbr, qps):
                for kb in kbr:
                    for mh in range(2):
                        nc.tensor.matmul(
                            qps[:, mh * 512:(mh + 1) * 512],
                            w_sb[kb][:, jt * 128:(jt + 1) * 128],
                            xview(cp, kb)[:, mh * 512:(mh + 1) * 512],
                            start=(kb == 0), stop=(kb == KB - 1))

            def emit_rotary(cp, jt, half, qps):
                c = cp * 2 + half
                ph = qps[:, half * 512:(half + 1) * 512]
                csl = cosT2[:, c * CW:(c + 1) * CW]
                ssl = sinT2[:, c * CW:(c + 1) * CW]
                t_sb = pst.tile([128, CW], BF16, tag="t_sb", bufs=3)
                nc.scalar.activation(t_sb[:], ph, AF.Copy)
                rps = psB.tile([128, CW], F32, tag="ps512",
                               name=f"rps_{cp}_{jt}_{half}")
                nc.tensor.matmul(rps[:], rmat[:], t_sb[:], start=True, stop=True)
                rp_sb = pst.tile([128, CW], BF16, tag="rp_sb", bufs=3)
                nc.scalar.activation(rp_sb[:], rps[:], AF.Copy)
                tmp = pst.tile([128, CW], BF16, tag="tmp", bufs=2)
                nc.vector.tensor_mul(tmp[:], t_sb[:], csl)
                rs = pst.tile([128, CW], BF16, tag="rs", bufs=2)
                nc.vector.tensor_mul(rs[:], rp_sb[:], ssl)
                if jt < 4:  # q or k -> straight into qT/kT
                    dst = qT[jt] if jt < 2 else kT[jt - 2]
                    nc.vector.tensor_add(dst[:, c * CW:(c + 1) * CW],
                                         tmp[:], rs[:])
                else:       # v -> rotate then transpose into V tiles
                    v_sb = pst.tile([128, CW], BF16, tag="v_sb", bufs=2)
                    nc.vector.tensor_add(v_sb[:], tmp[:], rs[:])
                    pair = jt - 4
                    vps = psB.tile([128, CW], BF16, tag="ps512",
                                   name=f"vps_{cp}_{pair}_{half}")
                    for rt in range(4):
                        nc.tensor.transpose(
                            vps[:, rt * 128:(rt + 1) * 128],
                            v_sb[:, rt * 128:(rt + 1) * 128],
                            identb[:])
                    vpsv = vps[:].rearrange("p (t hh d) -> p t hh d",
                                            t=4, hh=2)
                    for hh in range(2):
                        h = pair * 2 + hh
                        dstv = vt[h][:].rearrange(
                            "p (t c) -> p t c", c=DH + 1)[
                            :, c * 4:(c + 1) * 4, 0:DH]
                        nc.vector.tensor_copy(dstv, vpsv[:, :, hh, :])

            def emit_qkv(cp):
                for jt in (0, 2, 4, 1, 3, 5):
                    qps = psA.tile([128, 1024], F32, tag="psA",
                                   name=f"qps_{cp}_{jt}")
                    emit_qkv_mm(cp, jt, range(KB), qps)
                    for half in range(2):
                        emit_rotary(cp, jt, half, qps)

            def make_qkv_fillers(cp):
                out = []
                for jt in (0, 2, 4, 1, 3, 5):
                    st = {}
                    def fa(jt=jt, st=st):
                        st['qps'] = psA.tile([128, 1024], F32, tag="psA",
                                             name=f"qps_{cp}_{jt}")
                        emit_qkv_mm(cp, jt, range(0, 4), st['qps'])
                    def fb(jt=jt, st=st):
                        emit_qkv_mm(cp, jt, range(4, KB), st['qps'])
                    def fc(jt=jt, st=st):
                        emit_rotary(cp, jt, 0, st['qps'])
                    def fd(jt=jt, st=st):
                        emit_rotary(cp, jt, 1, st['qps'])
                    out += [fa, fb, fc, fd]
                return out

            # ---------------- phase 2: attention + projection ----------------
            fillers = []

            def fill_one():
                if fillers:
                    fillers.pop(0)()

            pending_norm = []

            def emit_norm2(h, av_t, s_rb, cc):
                pair, hh = h // 2, h % 2
                rbp = psB.tile([64, CW], F32, tag="ps512",
                               name=f"rbp_{h}_{cc}")
                nc.tensor.matmul(rbp[:], ones_b[0:1, 0:64], s_rb[:],
                                 start=True, stop=True)
                rb = pnm.tile([64, CW], F32, tag="rb", bufs=2,
                              name=f"rb_{h}_{cc}")
                nc.vector.reciprocal_approx_fast(rb[:], rbp[:])
                osl = oT[pair][hh * 64:(hh + 1) * 64, cc * CW:(cc + 1) * CW]
                nc.vector.tensor_mul(osl, av_t[0:DH, :], rb[:])

            def flush_norms():
                while pending_norm:
                    emit_norm2(*pending_norm.pop(0))

            def emit_attn_pair(c, hp):
                nj = 4 * c + 4          # j-blocks needed (causal)
                avs = {}
                for hh in range(2):
                    h = 2 * hp + hh
                    avs[h] = psB.tile([DH + 1, CW], F32, tag="ps512",
                                      name=f"av_{h}_{c}")
                for grp in range(nj // 2):
                    j0 = grp * 2
                    for hh in range(2):
                        h = 2 * hp + hh
                        pair = h // 2
                        qh = qT[pair][hh * 64:(hh + 1) * 64, :]
                        kh = kT[pair][hh * 64:(hh + 1) * 64, :]
                        sps = psA.tile([128, 1024], F32, tag="psA",
                                       name=f"sps_{h}_{c}_{grp}")
                        for g in range(2):
                            j = j0 + g
                            nc.tensor.matmul(
                                sps[:, g * 512:(g + 1) * 512],
                                kh[:, j * 128:(j + 1) * 128],
                                qh[:, c * CW:(c + 1) * CW],
                                start=True, stop=True)
                        pt = ppt.tile([128, 1024], BF16, tag="pt", bufs=6)
                        nc.scalar.activation(pt[:], sps[:], AF.Exp, scale=SCALE)
                        if j0 + 1 >= 4 * c:  # group touches the diagonal
                            w0 = min(CW, (j0 + 2 - 4 * c) * 128)
                            ptv = pt[:].rearrange("p (g i) -> p g i", g=2)[:, :, 0:w0]
                            nc.gpsimd.affine_select(
                                out=ptv, in_=ptv,
                                compare_op=OP.is_ge, fill=0.0,
                                base=c * CW - j0 * 128,
                                pattern=[[-128, 2], [1, w0]],
                                channel_multiplier=-1)
                        for g in range(2):
                            j = j0 + g
                            nc.tensor.matmul(avs[h][:],
                                             vt[h][:, j * (DH + 1):(j + 1) * (DH + 1)],
                                             pt[:, g * 512:(g + 1) * 512],
                                             start=(j == 0), stop=(j == nj - 1))
                        fill_one()
                for hh in range(2):
                    h = 2 * hp + hh
                    s_rb = pnm.tile([1, CW], BF16, tag="s_rb", bufs=2,
                                    name=f"s_rb_{h}_{c}")
                    nc.vector.tensor_copy(s_rb[:], avs[h][DH:DH + 1, :])
                    pending_norm.append((h, avs[h], s_rb, c))

            out_n = [0]

            def emit_proj_mm(nt_i, st):
                st['prj'] = psA.tile([128, DIM], F32, tag="psA",
                                     name=f"prj_{nt_i}")
                for mh in range(2):
                    for cb in range(2):
                        nc.tensor.matmul(
                            st['prj'][:, mh * 512:(mh + 1) * 512],
                            oT[cb][:, nt_i * 128:(nt_i + 1) * 128],
                            wo_sb[cb][:, mh * 512:(mh + 1) * 512],
                            start=(cb == 0), stop=(cb == 1))

            def emit_proj_fin(nt_i, st):
                ot = pout.tile([128, DIM], F16, tag="osb", bufs=3,
                               name=f"ot_{nt_i}")
                nc.vector.tensor_copy(ot[:], st['prj'][:])
                nc.sync.dma_start(outD[nt_i * 128:(nt_i + 1) * 128, :], ot[:])
                out_n[0] += 1

            def make_proj_fillers(cc):
                out = []
                for nt_i in range(4 * cc, 4 * cc + 4):
                    st = {}
                    out.append(lambda nt_i=nt_i, st=st: emit_proj_mm(nt_i, st))
                    out.append(lambda nt_i=nt_i, st=st: emit_proj_fin(nt_i, st))
                return out

            # ---------------- emission schedule ------------------------------
            emit_qkv(0)                     # q/k/v for n in [0, 1024)
            fillers += make_qkv_fillers(1)  # n in [1024, 2048) rides along
            for hp in range(2):
                flush_norms()
                emit_attn_pair(0, hp)       # 8 fill slots
            for hp in range(2):
                flush_norms()
                emit_attn_pair(1, hp)       # 16 fill slots
            while fillers:                  # make sure cp1 q/k/v is emitted
                fill_one()
            fillers += make_proj_fillers(0)
            fillers += make_proj_fillers(1)
            for hp in range(2):
                flush_norms()
                emit_attn_pair(2, hp)       # 24 fill slots
            while fillers:
                fill_one()
            fillers += make_proj_fillers(2)
            for hp in range(2):
                flush_norms()
                emit_attn_pair(3, hp)       # 32 fill slots
            flush_norms()
            while fillers:
                fill_one()
            for nt_i in range(12, 16):
                st = {}
                emit_proj_mm(nt_i, st)
                emit_proj_fin(nt_i, st)
            assert out_n[0] == NT, f"emitted {out_n[0]} proj tiles"

    nc.compile()
    return nc


def _get_program():
    if "nc" not in _CACHE:
        _CACHE["nc"] = _build_program()
    return _CACHE["nc"]


def _rot_lhsT():
    """lhsT for rot_half: out = lhsT.T @ tT = R @ tT, interleaved pairs."""
    R64 = np.zeros((64, 64), np.float32)
    for i in range(32):
        R64[2 * i, 2 * i + 1] = -1.0
        R64[2 * i + 1, 2 * i] = 1.0
    R = np.zeros((128, 128), np.float32)
    R[0:64, 0:64] = R64
    R[64:128, 64:128] = R64
    return np.ascontiguousarray(R.T)


def make_in_maps(x, rotary_pos_emb, w_qkv, w_out, b_out):
    x = np.asarray(x, np.float32)
    rotary_pos_emb = np.asarray(rotary_pos_emb, np.float32)
    w_qkv = np.asarray(w_qkv, np.float32)
    w_out = np.asarray(w_out, np.float32)

    import ml_dtypes
    bf16 = ml_dtypes.bfloat16
    identb = np.eye(128).astype(bf16)
    rmatT = _rot_lhsT()
    freqsT = np.ascontiguousarray(rotary_pos_emb.T)

    in_maps = []
    for c in range(NCORES):
        b = c // 4
        heads = [4 * (c % 4) + i for i in range(HPC)]
        # w_qkv column shard in j-tile order: q01,q23,k01,k23,v01,v23
        cols = []
        for t in range(3):            # q, k, v
            for h in heads:
                cols.append(w_qkv[:, t * H * DH + h * DH: t * H * DH + (h + 1) * DH])
        w_s = np.ascontiguousarray(np.concatenate(cols, axis=1))
        w_o = np.ascontiguousarray(
            np.concatenate([w_out[h * DH:(h + 1) * DH, :] for h in heads], axis=0))
        in_maps.append({
            "xT": np.ascontiguousarray(x[b].T).astype(bf16),
            "wqkv": w_s.astype(bf16),
            "wout": w_o.astype(bf16),
            "freqsT": freqsT,
            "rmatD": rmatT.astype(bf16),
            "identB": identb,
        })
    return in_maps


def kernel(x, rotary_pos_emb, w_qkv, w_out, b_out):
    from concourse.bass_utils import run_bass_kernel_spmd

    nc = _get_program()
    in_maps = make_in_maps(x, rotary_pos_emb, w_qkv, w_out, b_out)
    res = run_bass_kernel_spmd(nc, in_maps, list(range(NCORES))).results

    b_out = np.asarray(b_out, np.float32)
    out = np.zeros((B, N, DIM), np.float32)
    for c in range(NCORES):
        out[c // 4] += np.asarray(res[c]["out"], np.float32)
    out += b_out[None, None, :]
    return out


# revision 22
# speedup vs baseline: 1.2476x; 1.2476x over previous
"""Trainium2 Bass kernel for nn_Attention_43946105373274.

Causal multi-head attention with rotary embeddings applied to q, k and v.
B=2, N=2048, DIM=1024, H=16, DH=64, f32.

Sharding: 8 cores = (2 batches) x (4 head-groups of 4 heads).
Each core computes the qkv projection for its heads (w_qkv column-shard),
full causal attention for its heads, and a partial output projection
(w_out row-shard).  The host sums the 4 partials per batch and adds the
bias (the "all-reduce" of the output projection) — full inputs in, full
output out.

Per-core pipeline:
  0. x^T and freqs^T are transposed on the host (layout prep only), so
     all device DMAs are wide contiguous rows, batched into multi-tile
     transfers and spread across three DMA queues (sync/vector/scalar)
     so the first matmul can start ~5us in.  cos/sin are computed on
     device from freqs^T with a fused round-to-nearest range reduction;
     the pi/2 cosine shift rides the activation bias port.  No PE
     transposes and no fp32 PE instructions anywhere.
  1. QKV projection = bf16 matmuls accumulating in f32 PSUM, producing
     q/k/v in transposed [channels, n] layout (two heads per tile).
     Rotary: rotate_half via one bf16 R-matrix matmul; cos/sin combines
     on DVE in bf16.
  2. Attention is chunk-major with two heads interleaved, and a filler
     queue of independent tensor-engine work (the second half of the
     QKV projection, then output-projection tiles of the previous
     chunk) is popped between attention groups.  The PE therefore never
     idles while the scalar engine computes exp — idle gaps both waste
     cycles and demote the PE clock (2.4 -> 1.2 GHz) for multi-us
     epochs.  S^T = K-block^T q (f32 PSUM), exp on the scalar engine
     (DH^-0.5 folded into the exp scale; max-subtraction is
     mathematically redundant here), causal mask via gpsimd
     affine_select, AV accumulates with a ones-column appended to V so
     softmax row-sums fall out of the same matmul.  Normalization is
     split: the row-sum copy issues right after the last AV, the bf16
     ones-broadcast matmul + reciprocal + scale are deferred one head
     pair to stay off the critical path.
  3. Output-projection tiles ride the filler queue one chunk behind
     attention, so projection matmuls and the f16 output DMAs overlap
     the attention phase instead of serializing after it.
"""

import sys
import numpy as np

if "/opt/trn_rl_repo" not in sys.path:
    sys.path.insert(0, "/opt/trn_rl_repo")

B, N, DIM, H, DH = 2, 2048, 1024, 16, 64
HPC = 4                     # heads per core
NCORES = 8
SCALE = DH ** -0.5
NT = N // 128               # 16 row tiles
KB = DIM // 128             # 8 contraction blocks
CW = 512                    # i-chunk width
NCH = N // CW               # 4 chunks

_CACHE = {}


def _build_program():
    import concourse.bass as bass  # noqa: F401
    import concourse.mybir as mybir
    import concourse.tile as tile
    from concourse import bacc

    F32 = mybir.dt.float32
    F16 = mybir.dt.float16
    BF16 = mybir.dt.bfloat16
    AF = mybir.ActivationFunctionType
    OP = mybir.AluOpType

    nc = bacc.Bacc("TRN2", target_bir_lowering=False, debug=False,
                   num_devices=NCORES)

    xT = nc.dram_tensor("xT", [DIM, N], BF16, kind="ExternalInput")
    wqkv = nc.dram_tensor("wqkv", [DIM, 3 * HPC * DH], BF16, kind="ExternalInput")
    wout = nc.dram_tensor("wout", [HPC * DH, DIM], BF16, kind="ExternalInput")
    freqsT = nc.dram_tensor("freqsT", [DH, N], F32, kind="ExternalInput")
    rmatD = nc.dram_tensor("rmatD", [128, 128], BF16, kind="ExternalInput")
    identB = nc.dram_tensor("identB", [128, 128], BF16, kind="ExternalInput")
    outD = nc.dram_tensor("out", [N, DIM], F16, kind="ExternalOutput")

    MAGIC = 12582912.0          # 1.5 * 2**23: float32 round-to-nearest trick
    TWO_PI = float(2 * np.pi)
    INV_2PI = float(1.0 / TWO_PI)
    HALF_PI = float(np.pi / 2)

    with tile.TileContext(nc) as tc:
        with tc.tile_pool(name="pc", bufs=1) as pc, \
             tc.tile_pool(name="pw", bufs=1) as pw, \
             tc.tile_pool(name="px", bufs=8) as px, \
             tc.tile_pool(name="pqk", bufs=4) as pqk, \
             tc.tile_pool(name="pv", bufs=4) as pv, \
             tc.tile_pool(name="pst", bufs=3) as pst, \
             tc.tile_pool(name="ppt", bufs=6) as ppt, \
             tc.tile_pool(name="poT", bufs=2) as poT, \
             tc.tile_pool(name="pnm", bufs=2) as pnm, \
             tc.tile_pool(name="pout", bufs=3) as pout, \
             tc.tile_pool(name="ptr", bufs=1) as ptr, \
             tc.tile_pool(name="psA", bufs=2, space="PSUM") as psA, \
             tc.tile_pool(name="psB", bufs=3, space="PSUM") as psB:

            # ---------------- phase 0: DMAs ----------------------------------
            # The 16 DMA engines drain transfers in a single global FIFO, so
            # queue-splitting buys nothing — order one queue by criticality:
            # constants, w_qkv, x^T cp0, freqs, x^T cp1, w_out.
            rmat = pc.tile([128, 128], BF16, tag="rmat")
            nc.sync.dma_start(rmat[:], rmatD[:])
            identb = pc.tile([128, 128], BF16, tag="identb")
            nc.sync.dma_start(identb[:], identB[:])

            wall = pw.tile([128, KB, 3 * HPC * DH], BF16, tag="w")
            nc.sync.dma_start(
                wall[:], wqkv[:].rearrange("(kb p) j -> p kb j", p=128))
            w_sb = [wall[:, kb, :] for kb in range(KB)]

            xc = [px.tile([128, 2, 1024], BF16, tag="xT", name=f"xc_{cp}_{pr}")
                  for cp in range(2) for pr in range(4)]

            def xview(cp, kb):
                return xc[cp * 4 + kb // 2][:, kb % 2, :]

            def dma_xc(cp):
                for pr in range(4):     # pairs of kb
                    nc.sync.dma_start(
                        xc[cp * 4 + pr][:],
                        xT[pr * 256:(pr + 1) * 256,
                           cp * 1024:(cp + 1) * 1024].rearrange(
                               "(kb p) n -> p kb n", p=128))

            dma_xc(0)

            sarg = ptr.tile([128, N], F32, tag="sarg")
            carg = ptr.tile([128, N], F32, tag="carg")
            for half in range(2):
                nc.sync.dma_start(sarg[half * 64:(half + 1) * 64, :], freqsT[:])
                nc.sync.dma_start(carg[half * 64:(half + 1) * 64, :], freqsT[:])

            dma_xc(1)

            wot = pw.tile([128, 2, DIM], BF16, tag="wo")
            nc.sync.dma_start(
                wot[:], wout[:].rearrange("(cb p) d -> p cb d", p=128))
            wo_sb = [wot[:, cb, :] for cb in range(2)]

            ones_f = pc.tile([128, 128], F32, tag="ones_f")
            nc.vector.memset(ones_f[:], 1.0)
            ones_b = pc.tile([1, 128], BF16, tag="ones_b")
            nc.vector.tensor_copy(ones_b[:], ones_f[0:1, :])
            hpi_t = pc.tile([128, 1], F32, tag="hpi_t")
            nc.vector.memset(hpi_t[:], HALF_PI)
            zero_t = pc.tile([128, 1], F32, tag="zero_t")
            nc.vector.memset(zero_t[:], 0.0)

            # cos/sin in bf16, [128, N] (both 64-row halves identical)
            cosT2 = pc.tile([128, N], BF16, tag="cosT2")
            sinT2 = pc.tile([128, N], BF16, tag="sinT2")
            # sin(y) = Sin(y - 2pi*round(y/2pi))
            # cos(y) = Sin((y - 2pi*round(y/2pi + 1/4)) + pi/2)
            for cc in range(NCH):
                sl = slice(cc * CW, (cc + 1) * CW)
                for arg, dst, kofs, bias in (
                        (sarg, sinT2, 0.0, zero_t[:]),
                        (carg, cosT2, 0.25, hpi_t[:])):
                    k = pst.tile([128, CW], F32, tag="trk", bufs=2)
                    nc.vector.tensor_scalar(k[:], arg[:, sl], INV_2PI,
                                            MAGIC + kofs, op0=OP.mult,
                                            op1=OP.add)
                    nc.vector.tensor_scalar_sub(k[:], k[:], MAGIC)
                    xr = pst.tile([128, CW], F32, tag="trx", bufs=2)
                    nc.vector.scalar_tensor_tensor(xr[:], k[:], -TWO_PI,
                                                   arg[:, sl],
                                                   op0=OP.mult, op1=OP.add)
                    nc.scalar.activation(dst[:, sl], xr[:], AF.Sin, bias=bias)

            # persistent tensors
            qT = [pqk.tile([128, N], BF16, tag="qk", name=f"qT{i}") for i in range(2)]
            kT = [pqk.tile([128, N], BF16, tag="qk", name=f"kT{i}") for i in range(2)]
            # V tiles: [128, 65] per (head, row-tile); col 64 = ones
            vt = [pv.tile([128, NT * (DH + 1)], BF16, tag="v", name=f"vt{h}", bufs=4)
                  for h in range(HPC)]
            for h in range(HPC):
                vv = vt[h][:].rearrange("p (t c) -> p t c", c=DH + 1)
                nc.vector.tensor_copy(vv[:, :, DH:DH + 1],
                                      ones_f[:, 0:NT].unsqueeze(2))
            oT = [poT.tile([128, N], BF16, tag="oT", name=f"oT{i}") for i in range(2)]

            # ---------------- phase 1: qkv projection + rotary ---------------
            def emit_rotary(cp, jt, half, ph, pstag):
                c = cp * 2 + half
                csl = cosT2[:, c * CW:(c + 1) * CW]
                ssl = sinT2[:, c * CW:(c + 1) * CW]
                psbufs = 1 if pstag == "qp" else None
                copy_eng = nc.vector if pstag == "qp" else nc.scalar
                t_sb = pst.tile([128, CW], BF16, tag="t_sb", bufs=6)
                if copy_eng is nc.scalar:
                    nc.scalar.activation(t_sb[:], ph, AF.Copy)
                else:
                    nc.vector.tensor_copy(t_sb[:], ph)
                rps = psB.tile([128, CW], F32, tag=pstag, bufs=psbufs,
                               name=f"rps_{cp}_{jt}_{half}")
                nc.tensor.matmul(rps[:], rmat[:], t_sb[:], start=True, stop=True)
                tmp = pst.tile([128, CW], BF16, tag="tmp", bufs=3)
                nc.vector.tensor_mul(tmp[:], t_sb[:], csl)
                rs = pst.tile([128, CW], BF16, tag="rs", bufs=3)
                nc.vector.tensor_mul(rs[:], rps[:], ssl)
                if jt < 4:  # q or k -> straight into qT/kT
                    dst = qT[jt] if jt < 2 else kT[jt - 2]
                    nc.vector.tensor_add(dst[:, c * CW:(c + 1) * CW],
                                         tmp[:], rs[:])
                else:       # v -> rotate then transpose into V tiles
                    v_sb = pst.tile([128, CW], BF16, tag="v_sb", bufs=2)
                    nc.vector.tensor_add(v_sb[:], tmp[:], rs[:])
                    pair = jt - 4
                    vps = psB.tile([128, CW], BF16, tag=pstag, bufs=psbufs,
                                   name=f"vps_{cp}_{pair}_{half}")
                    for rt in range(4):
                        nc.tensor.transpose(
                            vps[:, rt * 128:(rt + 1) * 128],
                            v_sb[:, rt * 128:(rt + 1) * 128],
                            identb[:])
                    vpsv = vps[:].rearrange("p (t hh d) -> p t hh d",
                                            t=4, hh=2)
                    for hh in range(2):
                        h = pair * 2 + hh
                        dstv = vt[h][:].rearrange(
                            "p (t c) -> p t c", c=DH + 1)[
                            :, c * 4:(c + 1) * 4, 0:DH]
                        nc.vector.tensor_copy(dstv, vpsv[:, :, hh, :])

            def emit_qkv(cp):
                for jt in (0, 2, 4, 1, 3, 5):
                    qps = psA.tile([128, 1024], F32, tag="psA",
                                   name=f"qps_{cp}_{jt}")
                    for kb in range(KB):
                        for mh in range(2):
                            nc.tensor.matmul(
                                qps[:, mh * 512:(mh + 1) * 512],
                                w_sb[kb][:, jt * 128:(jt + 1) * 128],
                                xview(cp, kb)[:, mh * 512:(mh + 1) * 512],
                                start=(kb == 0), stop=(kb == KB - 1))
                    for half in range(2):
                        emit_rotary(cp, jt, half,
                                    qps[:, half * 512:(half + 1) * 512],
                                    "ps512")

            def make_qkv_fillers(cp):
                # Each (jt, half) is two pops: the 512-wide projection into a
                # dedicated 2KB PSUM slot (tag "qp", never contended by the
                # attention accumulators), then the rotary combine.
                out = []
                for jt in (0, 2, 4, 1, 3, 5):
                    for half in range(2):
                        st = {}
                        def fa(jt=jt, half=half, st=st):
                            st['qp'] = psB.tile([128, CW], F32, tag="qp",
                                                bufs=1,
                                                name=f"qp_{cp}_{jt}_{half}")
                            for kb in range(KB):
                                nc.tensor.matmul(
                                    st['qp'][:],
                                    w_sb[kb][:, jt * 128:(jt + 1) * 128],
                                    xview(cp, kb)[:, half * 512:(half + 1) * 512],
                                    start=(kb == 0), stop=(kb == KB - 1))
                        def fb(jt=jt, half=half, st=st):
                            emit_rotary(cp, jt, half, st['qp'][:], "qp")
                        out += [fa, fb]
                return out

            # ---------------- phase 2: attention + projection ----------------
            fillers = []

            def fill_one():
                if fillers:
                    fillers.pop(0)()

            pending_norm = []

            def emit_norm2(h, av_t, s_rb, cc):
                pair, hh = h // 2, h % 2
                rbp = psB.tile([64, CW], F32, tag="ps512",
                               name=f"rbp_{h}_{cc}")
                nc.tensor.matmul(rbp[:], ones_b[0:1, 0:64], s_rb[:],
                                 start=True, stop=True)
                rb = pnm.tile([64, CW], F32, tag="rb", bufs=2,
                              name=f"rb_{h}_{cc}")
                nc.vector.reciprocal_approx_fast(rb[:], rbp[:])
                osl = oT[pair][hh * 64:(hh + 1) * 64, cc * CW:(cc + 1) * CW]
                nc.vector.tensor_mul(osl, av_t[0:DH, :], rb[:])

            def flush_norms():
                while pending_norm:
                    emit_norm2(*pending_norm.pop(0))

            def emit_attn_pair(c, hp, fill_every=1):
                it = [0]
                nj = 4 * c + 4          # j-blocks needed (causal)
                avs = {}
                for hh in range(2):
                    h = 2 * hp + hh
                    avs[h] = psB.tile([DH + 1, CW], F32, tag="ps512",
                                      name=f"av_{h}_{c}")
                for grp in range(nj // 2):
                    j0 = grp * 2
                    diag = j0 + 1 >= 4 * c   # group touches the diagonal
                    for hh in range(2):
                        h = 2 * hp + hh
                        pair = h // 2
                        qh = qT[pair][hh * 64:(hh + 1) * 64, :]
                        kh = kT[pair][hh * 64:(hh + 1) * 64, :]
                        sps = psA.tile([128, 1024], F32, tag="psA",
                                       name=f"sps_{h}_{c}_{grp}")
                        pt = ppt.tile([128, 1024], BF16, tag="pt", bufs=6)
                        if not diag:
                            for g in range(2):
                                j = j0 + g
                                nc.tensor.matmul(
                                    sps[:, g * 512:(g + 1) * 512],
                                    kh[:, j * 128:(j + 1) * 128],
                                    qh[:, c * CW:(c + 1) * CW],
                                    start=True, stop=True)
                            nc.scalar.activation(pt[:], sps[:], AF.Exp,
                                                 scale=SCALE)
                            for g in range(2):
                                j = j0 + g
                                nc.tensor.matmul(
                                    avs[h][:],
                                    vt[h][:, j * (DH + 1):(j + 1) * (DH + 1)],
                                    pt[:, g * 512:(g + 1) * 512],
                                    start=(j == 0), stop=(j == nj - 1))
                        else:
                            # diagonal blocks: columns i < goff are fully
                            # masked — skip their S/exp/AV entirely; only a
                            # fixed 128-wide triangle needs affine_select.
                            for g in range(2):
                                j = j0 + g
                                goff = (j - 4 * c) * 128
                                sl = slice(g * 512 + goff, (g + 1) * 512)
                                nc.tensor.matmul(
                                    sps[:, sl],
                                    kh[:, j * 128:(j + 1) * 128],
                                    qh[:, c * CW + goff:(c + 1) * CW],
                                    start=True, stop=True)
                                nc.scalar.activation(pt[:, sl], sps[:, sl],
                                                     AF.Exp, scale=SCALE)
                                tri = pt[:, g * 512 + goff:
                                         g * 512 + goff + 128]
                                nc.gpsimd.affine_select(
                                    out=tri, in_=tri,
                                    compare_op=OP.is_ge, fill=0.0,
                                    base=0, pattern=[[1, 128]],
                                    channel_multiplier=-1)
                            for g in range(2):
                                j = j0 + g
                                goff = (j - 4 * c) * 128
                                nc.tensor.matmul(
                                    avs[h][:, goff:],
                                    vt[h][:, j * (DH + 1):(j + 1) * (DH + 1)],
                                    pt[:, g * 512 + goff:(g + 1) * 512],
                                    start=(j == 0), stop=(j == nj - 1),
                                    skip_group_check=True)
                        if it[0] % fill_every == 0:
                            fill_one()
                        it[0] += 1
                for hh in range(2):
                    h = 2 * hp + hh
                    s_rb = pnm.tile([1, CW], BF16, tag="s_rb", bufs=2,
                                    name=f"s_rb_{h}_{c}")
                    nc.vector.tensor_copy(s_rb[:], avs[h][DH:DH + 1, :])
                    pending_norm.append((h, avs[h], s_rb, c))

            out_n = [0]

            def emit_proj_mm(nt_i, st):
                st['prj'] = psA.tile([128, DIM], F32, tag="psA",
                                     name=f"prj_{nt_i}")
                for mh in range(2):
                    for cb in range(2):
                        nc.tensor.matmul(
                            st['prj'][:, mh * 512:(mh + 1) * 512],
                            oT[cb][:, nt_i * 128:(nt_i + 1) * 128],
                            wo_sb[cb][:, mh * 512:(mh + 1) * 512],
                            start=(cb == 0), stop=(cb == 1))

            def emit_proj_fin(nt_i, st):
                ot = pout.tile([128, DIM], F16, tag="osb", bufs=3,
                               name=f"ot_{nt_i}")
                nc.vector.tensor_copy(ot[:], st['prj'][:])
                nc.sync.dma_start(outD[nt_i * 128:(nt_i + 1) * 128, :], ot[:])
                out_n[0] += 1

            def make_proj_fillers(cc):
                def one(nt_i):
                    st = {}
                    emit_proj_mm(nt_i, st)
                    emit_proj_fin(nt_i, st)
                return [lambda nt_i=nt_i: one(nt_i)
                        for nt_i in range(4 * cc, 4 * cc + 4)]

            # ---------------- emission schedule ------------------------------
            emit_qkv(0)                     # q/k/v for n in [0, 1024)
            fillers += make_qkv_fillers(1)  # n in [1024, 2048) rides along
            for hp in range(2):
                flush_norms()
                emit_attn_pair(0, hp)       # 8 fill slots
            for hp in range(2):
                flush_norms()
                emit_attn_pair(1, hp)       # 16 fill slots
            while fillers:                  # make sure cp1 q/k/v is emitted
                fill_one()
            fillers += make_proj_fillers(0)
            for hp in range(2):
                flush_norms()
                emit_attn_pair(2, hp, fill_every=5)   # 24 slots, 4 fillers
            while fillers:
                fill_one()
            fillers += make_proj_fillers(1)
            fillers += make_proj_fillers(2)
            for hp in range(2):
                flush_norms()
                emit_attn_pair(3, hp, fill_every=4)   # 32 slots, 8 fillers
            flush_norms()
            while fillers:
                fill_one()
            for nt_i in range(12, 16):
                st = {}
                emit_proj_mm(nt_i, st)
                emit_proj_fin(nt_i, st)
            assert out_n[0] == NT, f"emitted {out_n[0]} proj tiles"

    nc.compile()
    return nc


def _get_program():
    if "nc" not in _CACHE:
        _CACHE["nc"] = _build_program()
    return _CACHE["nc"]


def _rot_lhsT():
    """lhsT for rot_half: out = lhsT.T @ tT = R @ tT, interleaved pairs."""
    R64 = np.zeros((64, 64), np.float32)
    for i in range(32):
        R64[2 * i, 2 * i + 1] = -1.0
        R64[2 * i + 1, 2 * i] = 1.0
    R = np.zeros((128, 128), np.float32)
    R[0:64, 0:64] = R64
    R[64:128, 64:128] = R64
    return np.ascontiguousarray(R.T)


def make_in_maps(x, rotary_pos_emb, w_qkv, w_out, b_out):
    x = np.asarray(x, np.float32)
    rotary_pos_emb = np.asarray(rotary_pos_emb, np.float32)
    w_qkv = np.asarray(w_qkv, np.float32)
    w_out = np.asarray(w_out, np.float32)

    import ml_dtypes
    bf16 = ml_dtypes.bfloat16
    identb = np.eye(128).astype(bf16)
    rmatT = _rot_lhsT()
    freqsT = np.ascontiguousarray(rotary_pos_emb.T)

    in_maps = []
    for c in range(NCORES):
        b = c // 4
        heads = [4 * (c % 4) + i for i in range(HPC)]
        # w_qkv column shard in j-tile order: q01,q23,k01,k23,v01,v23
        cols = []
        for t in range(3):            # q, k, v
            for h in heads:
                cols.append(w_qkv[:, t * H * DH + h * DH: t * H * DH + (h + 1) * DH])
        w_s = np.ascontiguousarray(np.concatenate(cols, axis=1))
        w_o = np.ascontiguousarray(
            np.concatenate([w_out[h * DH:(h + 1) * DH, :] for h in heads], axis=0))
        in_maps.append({
            "xT": np.ascontiguousarray(x[b].T).astype(bf16),
            "wqkv": w_s.astype(bf16),
            "wout": w_o.astype(bf16),
            "freqsT": freqsT,
            "rmatD": rmatT.astype(bf16),
            "identB": identb,
        })
    return in_maps


def kernel(x, rotary_pos_emb, w_qkv, w_out, b_out):
    from concourse.bass_utils import run_bass_kernel_spmd

    nc = _get_program()
    in_maps = make_in_maps(x, rotary_pos_emb, w_qkv, w_out, b_out)
    res = run_bass_kernel_spmd(nc, in_maps, list(range(NCORES))).results

    b_out = np.asarray(b_out, np.float32)
    out = np.zeros((B, N, DIM), np.float32)
    for c in range(NCORES):
        out[c // 4] += np.asarray(res[c]["out"], np.float32)
    out += b_out[None, None, :]
    return out


# revision 48
# speedup vs baseline: 1.2622x; 1.0117x over previous
"""Trainium2 Bass kernel for nn_Attention_43946105373274.

Causal multi-head attention with rotary embeddings applied to q, k and v.
B=2, N=2048, DIM=1024, H=16, DH=64, f32.

Sharding: 8 cores = (2 batches) x (4 head-groups of 4 heads).
Each core computes the qkv projection for its heads (w_qkv column-shard),
full causal attention for its heads, and a partial output projection
(w_out row-shard).  The host sums the 4 partials per batch and adds the
bias (the "all-reduce" of the output projection) — full inputs in, full
output out.

Per-core pipeline:
  0. x^T and freqs^T are transposed on the host (layout prep only), so
     all device DMAs are wide contiguous rows, batched into multi-tile
     transfers and spread across three DMA queues (sync/vector/scalar)
     so the first matmul can start ~5us in.  cos/sin are computed on
     device from freqs^T with a fused round-to-nearest range reduction;
     the pi/2 cosine shift rides the activation bias port.  No PE
     transposes and no fp32 PE instructions anywhere.
  1. QKV projection = bf16 matmuls accumulating in f32 PSUM, producing
     q/k/v in transposed [channels, n] layout (two heads per tile).
     Rotary: rotate_half via one bf16 R-matrix matmul; cos/sin combines
     on DVE in bf16.
  2. Attention is chunk-major with two heads interleaved, and a filler
     queue of independent tensor-engine work (the second half of the
     QKV projection, then output-projection tiles of the previous
     chunk) is popped between attention groups.  The PE therefore never
     idles while the scalar engine computes exp — idle gaps both waste
     cycles and demote the PE clock (2.4 -> 1.2 GHz) for multi-us
     epochs.  S^T = K-block^T q (f32 PSUM), exp on the scalar engine
     (DH^-0.5 folded into the exp scale; max-subtraction is
     mathematically redundant here), causal mask via gpsimd
     affine_select, AV accumulates with a ones-column appended to V so
     softmax row-sums fall out of the same matmul.  Normalization is
     split: the row-sum copy issues right after the last AV, the bf16
     ones-broadcast matmul + reciprocal + scale are deferred one head
     pair to stay off the critical path.
  3. Output-projection tiles ride the filler queue one chunk behind
     attention, so projection matmuls and the f16 output DMAs overlap
     the attention phase instead of serializing after it.
"""

import sys
import numpy as np

if "/opt/trn_rl_repo" not in sys.path:
    sys.path.insert(0, "/opt/trn_rl_repo")

B, N, DIM, H, DH = 2, 2048, 1024, 16, 64
HPC = 4                     # heads per core
NCORES = 8
SCALE = DH ** -0.5
NT = N // 128               # 16 row tiles
KB = DIM // 128             # 8 contraction blocks
CW = 512                    # i-chunk width
NCH = N // CW               # 4 chunks

_CACHE = {}


def _build_program():
    import concourse.bass as bass  # noqa: F401
    import concourse.mybir as mybir
    import concourse.tile as tile
    from concourse import bacc

    F32 = mybir.dt.float32
    F16 = mybir.dt.float16
    BF16 = mybir.dt.bfloat16
    AF = mybir.ActivationFunctionType
    OP = mybir.AluOpType

    nc = bacc.Bacc("TRN2", target_bir_lowering=False, debug=False,
                   num_devices=NCORES)

    xT = nc.dram_tensor("xT", [DIM, N], BF16, kind="ExternalInput")
    wqkv = nc.dram_tensor("wqkv", [DIM, 3 * HPC * DH], BF16, kind="ExternalInput")
    wout = nc.dram_tensor("wout", [HPC * DH, DIM], BF16, kind="ExternalInput")
    freqsT = nc.dram_tensor("freqsT", [DH, N], F32, kind="ExternalInput")
    rmatD = nc.dram_tensor("rmatD", [128, 128], BF16, kind="ExternalInput")
    identB = nc.dram_tensor("identB", [128, 128], BF16, kind="ExternalInput")
    outD = nc.dram_tensor("out", [N, DIM], F16, kind="ExternalOutput")

    MAGIC = 12582912.0          # 1.5 * 2**23: float32 round-to-nearest trick
    TWO_PI = float(2 * np.pi)
    INV_2PI = float(1.0 / TWO_PI)
    HALF_PI = float(np.pi / 2)

    with tile.TileContext(nc) as tc:
        with tc.tile_pool(name="pc", bufs=1) as pc, \
             tc.tile_pool(name="pw", bufs=1) as pw, \
             tc.tile_pool(name="px", bufs=8) as px, \
             tc.tile_pool(name="pqk", bufs=4) as pqk, \
             tc.tile_pool(name="pv", bufs=4) as pv, \
             tc.tile_pool(name="pst", bufs=3) as pst, \
             tc.tile_pool(name="ppt", bufs=6) as ppt, \
             tc.tile_pool(name="poT", bufs=2) as poT, \
             tc.tile_pool(name="pnm", bufs=2) as pnm, \
             tc.tile_pool(name="pout", bufs=3) as pout, \
             tc.tile_pool(name="ptr", bufs=1) as ptr, \
             tc.tile_pool(name="psA", bufs=2, space="PSUM") as psA, \
             tc.tile_pool(name="psB", bufs=3, space="PSUM") as psB:

            # ---------------- phase 0: DMAs ----------------------------------
            # The 16 DMA engines drain transfers in a single global FIFO, so
            # queue-splitting buys nothing — order one queue by criticality:
            # constants, w_qkv, x^T cp0, freqs, x^T cp1, w_out.
            rmat = pc.tile([128, 128], BF16, tag="rmat")
            nc.sync.dma_start(rmat[:], rmatD[:])
            identb = pc.tile([128, 128], BF16, tag="identb")
            nc.sync.dma_start(identb[:], identB[:])

            wall = pw.tile([128, KB, 3 * HPC * DH], BF16, tag="w")
            for wh in range(2):
                nc.sync.dma_start(
                    wall[:, wh * 4:(wh + 1) * 4, :],
                    wqkv[wh * 512:(wh + 1) * 512, :].rearrange(
                        "(kb p) j -> p kb j", p=128))
            w_sb = [wall[:, kb, :] for kb in range(KB)]

            xc = [px.tile([128, 2, 1024], BF16, tag="xT", name=f"xc_{cp}_{pr}")
                  for cp in range(2) for pr in range(4)]

            def xview(cp, kb):
                return xc[cp * 4 + kb // 2][:, kb % 2, :]

            def dma_xc(cp, prs):
                for pr in prs:          # pairs of kb
                    nc.sync.dma_start(
                        xc[cp * 4 + pr][:],
                        xT[pr * 256:(pr + 1) * 256,
                           cp * 1024:(cp + 1) * 1024].rearrange(
                               "(kb p) n -> p kb n", p=128))

            dma_xc(0, range(2))

            sarg = ptr.tile([128, N], F32, tag="sarg")
            for half in range(2):
                nc.sync.dma_start(sarg[half * 64:(half + 1) * 64, :], freqsT[:])

            dma_xc(0, range(2, 4))
            dma_xc(1, range(4))

            wot = pw.tile([128, 2, DIM], BF16, tag="wo")
            nc.sync.dma_start(
                wot[:], wout[:].rearrange("(cb p) d -> p cb d", p=128))
            wo_sb = [wot[:, cb, :] for cb in range(2)]

            ones_f = pc.tile([128, 128], F32, tag="ones_f")
            nc.vector.memset(ones_f[:], 1.0)
            ones_b = pc.tile([1, 128], BF16, tag="ones_b")
            nc.vector.tensor_copy(ones_b[:], ones_f[0:1, :])
            hpi_t = pc.tile([128, 1], F32, tag="hpi_t")
            nc.vector.memset(hpi_t[:], HALF_PI)
            zero_t = pc.tile([128, 1], F32, tag="zero_t")
            nc.vector.memset(zero_t[:], 0.0)

            # cos/sin in bf16, [128, N] (both 64-row halves identical).
            # Range-reduce to [-pi, pi] for the Sin spline; the
            # round-to-nearest (magic-constant) steps ride the scalar
            # engine's scale+bias ports so only one DVE op remains.
            # sin(y) = Sin(y - 2pi*round(y/2pi))
            # cos(y) = Sin((y - 2pi*round(y/2pi + 1/4)) + pi/2)
            cosT2 = pc.tile([128, N], BF16, tag="cosT2")
            sinT2 = pc.tile([128, N], BF16, tag="sinT2")
            for cc in range(NCH):
                sl = slice(cc * CW, (cc + 1) * CW)
                for dst, kofs, bias in ((sinT2, 0.0, zero_t[:]),
                                        (cosT2, 0.25, hpi_t[:])):
                    k = pst.tile([128, CW], F32, tag="trk", bufs=2)
                    nc.scalar.activation(k[:], sarg[:, sl], AF.Copy,
                                         scale=INV_2PI, bias=MAGIC + kofs)
                    nc.scalar.activation(k[:], k[:], AF.Copy, bias=-MAGIC)
                    xr = pst.tile([128, CW], F32, tag="trx", bufs=2)
                    nc.vector.scalar_tensor_tensor(xr[:], k[:], -TWO_PI,
                                                   sarg[:, sl],
                                                   op0=OP.mult, op1=OP.add)
                    nc.scalar.activation(dst[:, sl], xr[:], AF.Sin, bias=bias)

            # persistent tensors
            qT = [pqk.tile([128, N], BF16, tag="qk", name=f"qT{i}") for i in range(2)]
            kT = [pqk.tile([128, N], BF16, tag="qk", name=f"kT{i}") for i in range(2)]
            # V tiles: [128, 65] per (head, row-tile); col 64 = ones
            vt = [pv.tile([128, NT * (DH + 1)], BF16, tag="v", name=f"vt{h}", bufs=4)
                  for h in range(HPC)]
            for h in range(HPC):
                vv = vt[h][:].rearrange("p (t c) -> p t c", c=DH + 1)
                nc.vector.tensor_copy(vv[:, :, DH:DH + 1],
                                      ones_f[:, 0:NT].unsqueeze(2))
            oT = [poT.tile([128, N], BF16, tag="oT", name=f"oT{i}") for i in range(2)]

            # ---------------- phase 1: qkv projection + rotary ---------------
            def emit_rotary(cp, jt, half, ph, pstag):
                c = cp * 2 + half
                csl = cosT2[:, c * CW:(c + 1) * CW]
                ssl = sinT2[:, c * CW:(c + 1) * CW]
                psbufs = 1 if pstag == "qp" else None
                copy_eng = nc.vector if pstag == "qp" else nc.scalar
                t_sb = pst.tile([128, CW], BF16, tag="t_sb", bufs=6)
                if copy_eng is nc.scalar:
                    nc.scalar.activation(t_sb[:], ph, AF.Copy)
                else:
                    nc.vector.tensor_copy(t_sb[:], ph)
                rps = psB.tile([128, CW], F32, tag=pstag, bufs=psbufs,
                               name=f"rps_{cp}_{jt}_{half}")
                nc.tensor.matmul(rps[:], rmat[:], t_sb[:], start=True, stop=True)
                tmp = pst.tile([128, CW], BF16, tag="tmp", bufs=3)
                nc.vector.tensor_mul(tmp[:], t_sb[:], csl)
                rs = pst.tile([128, CW], BF16, tag="rs", bufs=3)
                nc.vector.tensor_mul(rs[:], rps[:], ssl)
                if jt < 4:  # q or k -> straight into qT/kT
                    dst = qT[jt] if jt < 2 else kT[jt - 2]
                    nc.vector.tensor_add(dst[:, c * CW:(c + 1) * CW],
                                         tmp[:], rs[:])
                else:       # v -> rotate then transpose into V tiles
                    v_sb = pst.tile([128, CW], BF16, tag="v_sb", bufs=2)
                    nc.vector.tensor_add(v_sb[:], tmp[:], rs[:])
                    pair = jt - 4
                    vps = psB.tile([128, CW], BF16, tag=pstag, bufs=psbufs,
                                   name=f"vps_{cp}_{pair}_{half}")
                    for rt in range(4):
                        nc.tensor.transpose(
                            vps[:, rt * 128:(rt + 1) * 128],
                            v_sb[:, rt * 128:(rt + 1) * 128],
                            identb[:])
                    vpsv = vps[:].rearrange("p (t hh d) -> p t hh d",
                                            t=4, hh=2)
                    for hh in range(2):
                        h = pair * 2 + hh
                        dstv = vt[h][:].rearrange(
                            "p (t c) -> p t c", c=DH + 1)[
                            :, c * 4:(c + 1) * 4, 0:DH]
                        nc.vector.tensor_copy(dstv, vpsv[:, :, hh, :])

            def qkv_mm(cp, jt, qps, kbr):
                for kb in kbr:
                    for mh in range(2):
                        nc.tensor.matmul(
                            qps[:, mh * 512:(mh + 1) * 512],
                            w_sb[kb][:, jt * 128:(jt + 1) * 128],
                            xview(cp, kb)[:, mh * 512:(mh + 1) * 512],
                            start=(kb == 0), stop=(kb == KB - 1))

            def emit_qkv_jt(cp, jt):
                qps = psA.tile([128, 1024], F32, tag="psA",
                               name=f"qps_{cp}_{jt}")
                qkv_mm(cp, jt, qps, range(KB))
                for half in range(2):
                    emit_rotary(cp, jt, half,
                                qps[:, half * 512:(half + 1) * 512], "ps512")

            def emit_qkv_pair_interleaved(cp, jta, jtb):
                # kb halves interleaved across two jts so the tensor engine
                # has runway while the later x^T pairs are still in flight
                qa = psA.tile([128, 1024], F32, tag="psA", name=f"qps_{cp}_{jta}")
                qkv_mm(cp, jta, qa, range(0, 4))
                qb = psA.tile([128, 1024], F32, tag="psA", name=f"qps_{cp}_{jtb}")
                qkv_mm(cp, jtb, qb, range(0, 4))
                qkv_mm(cp, jta, qa, range(4, KB))
                for half in range(2):
                    emit_rotary(cp, jta, half,
                                qa[:, half * 512:(half + 1) * 512], "ps512")
                qkv_mm(cp, jtb, qb, range(4, KB))
                for half in range(2):
                    emit_rotary(cp, jtb, half,
                                qb[:, half * 512:(half + 1) * 512], "ps512")

            def make_qkv_fillers(cp, jts=(0, 2, 4, 1, 3, 5)):
                # Each (jt, half) is two pops: the 512-wide projection into a
                # dedicated 2KB PSUM slot (tag "qp", never contended by the
                # attention accumulators), then the rotary combine.
                out = []
                for jt in jts:
                    for half in range(2):
                        st = {}
                        def fa(jt=jt, half=half, st=st):
                            st['qp'] = psB.tile([128, CW], F32, tag="qp",
                                                bufs=1,
                                                name=f"qp_{cp}_{jt}_{half}")
                            for kb in range(KB):
                                nc.tensor.matmul(
                                    st['qp'][:],
                                    w_sb[kb][:, jt * 128:(jt + 1) * 128],
                                    xview(cp, kb)[:, half * 512:(half + 1) * 512],
                                    start=(kb == 0), stop=(kb == KB - 1))
                        def fb(jt=jt, half=half, st=st):
                            emit_rotary(cp, jt, half, st['qp'][:], "qp")
                        out += [fa, fb]
                return out

            # ---------------- phase 2: attention + projection ----------------
            fillers = []

            def fill_one():
                if fillers:
                    fillers.pop(0)()

            pending_norm = []

            def emit_norm2(h, av_t, s_rb, cc):
                pair, hh = h // 2, h % 2
                rbp = psB.tile([64, CW], F32, tag="ps512",
                               name=f"rbp_{h}_{cc}")
                nc.tensor.matmul(rbp[:], ones_b[0:1, 0:64], s_rb[:],
                                 start=True, stop=True)
                rb = pnm.tile([64, CW], F32, tag="rb", bufs=2,
                              name=f"rb_{h}_{cc}")
                nc.vector.reciprocal_approx_fast(rb[:], rbp[:])
                osl = oT[pair][hh * 64:(hh + 1) * 64, cc * CW:(cc + 1) * CW]
                nc.vector.tensor_mul(osl, av_t[0:DH, :], rb[:])

            def flush_norms():
                while pending_norm:
                    emit_norm2(*pending_norm.pop(0))

            def emit_attn_pair(c, hp, fill_every=1):
                it = [0]
                nj = 4 * c + 4          # j-blocks needed (causal)
                avs = {}
                for hh in range(2):
                    h = 2 * hp + hh
                    avs[h] = psB.tile([DH + 1, CW], F32, tag="ps512",
                                      name=f"av_{h}_{c}")
                # AV matmuls run two iterations behind their exp so the
                # in-order tensor engine never waits on the scalar engine.
                pend_av = []

                def pop_av():
                    pend_av.pop(0)()

                for grp in range(nj // 2):
                    j0 = grp * 2
                    diag = j0 + 1 >= 4 * c   # group touches the diagonal
                    for hh in range(2):
                        h = 2 * hp + hh
                        pair = h // 2
                        qh = qT[pair][hh * 64:(hh + 1) * 64, :]
                        kh = kT[pair][hh * 64:(hh + 1) * 64, :]
                        sps = psA.tile([128, 1024], F32, tag="psA",
                                       name=f"sps_{h}_{c}_{grp}")
                        pt = ppt.tile([128, 1024], BF16, tag="pt", bufs=6)
                        if not diag:
                            for g in range(2):
                                j = j0 + g
                                nc.tensor.matmul(
                                    sps[:, g * 512:(g + 1) * 512],
                                    kh[:, j * 128:(j + 1) * 128],
                                    qh[:, c * CW:(c + 1) * CW],
                                    start=True, stop=True)
                            nc.scalar.activation(pt[:], sps[:], AF.Exp,
                                                 scale=SCALE)

                            def do_av(h=h, j0=j0, pt=pt):
                                for g in range(2):
                                    j = j0 + g
                                    nc.tensor.matmul(
                                        avs[h][:],
                                        vt[h][:, j * (DH + 1):(j + 1) * (DH + 1)],
                                        pt[:, g * 512:(g + 1) * 512],
                                        start=(j == 0), stop=(j == nj - 1))
                        else:
                            # diagonal blocks: columns i < goff are fully
                            # masked — skip their S/exp/AV entirely; only a
                            # fixed 128-wide triangle needs affine_select.
                            for g in range(2):
                                j = j0 + g
                                goff = (j - 4 * c) * 128
                                sl = slice(g * 512 + goff, (g + 1) * 512)
                                nc.tensor.matmul(
                                    sps[:, sl],
                                    kh[:, j * 128:(j + 1) * 128],
                                    qh[:, c * CW + goff:(c + 1) * CW],
                                    start=True, stop=True)
                                nc.scalar.activation(pt[:, sl], sps[:, sl],
                                                     AF.Exp, scale=SCALE)
                                tri = pt[:, g * 512 + goff:
                                         g * 512 + goff + 128]
                                nc.gpsimd.affine_select(
                                    out=tri, in_=tri,
                                    compare_op=OP.is_ge, fill=0.0,
                                    base=0, pattern=[[1, 128]],
                                    channel_multiplier=-1)

                            def do_av(h=h, j0=j0, pt=pt, c=c):
                                for g in range(2):
                                    j = j0 + g
                                    goff = (j - 4 * c) * 128
                                    nc.tensor.matmul(
                                        avs[h][:, goff:],
                                        vt[h][:, j * (DH + 1):(j + 1) * (DH + 1)],
                                        pt[:, g * 512 + goff:(g + 1) * 512],
                                        start=(j == 0), stop=(j == nj - 1),
                                        skip_group_check=True)
                        pend_av.append(do_av)
                        if len(pend_av) > 2:
                            pop_av()
                        if it[0] % fill_every == 0:
                            fill_one()
                        it[0] += 1
                while pend_av:
                    pop_av()
                for hh in range(2):
                    h = 2 * hp + hh
                    s_rb = pnm.tile([1, CW], BF16, tag="s_rb", bufs=2,
                                    name=f"s_rb_{h}_{c}")
                    nc.vector.tensor_copy(s_rb[:], avs[h][DH:DH + 1, :])
                    pending_norm.append((h, avs[h], s_rb, c))

            out_n = [0]

            def emit_proj_mm(nt_i, st):
                st['prj'] = psA.tile([128, DIM], F32, tag="psA",
                                     name=f"prj_{nt_i}")
                for mh in range(2):
                    for cb in range(2):
                        nc.tensor.matmul(
                            st['prj'][:, mh * 512:(mh + 1) * 512],
                            oT[cb][:, nt_i * 128:(nt_i + 1) * 128],
                            wo_sb[cb][:, mh * 512:(mh + 1) * 512],
                            start=(cb == 0), stop=(cb == 1))

            def emit_proj_fin(nt_i, st):
                ot = pout.tile([128, DIM], F16, tag="osb", bufs=3,
                               name=f"ot_{nt_i}")
                nc.vector.tensor_copy(ot[:], st['prj'][:])
                eng = nc.scalar if nt_i % 2 else nc.sync
                eng.dma_start(outD[nt_i * 128:(nt_i + 1) * 128, :], ot[:])
                out_n[0] += 1

            def make_proj_fillers(cc):
                def one(nt_i):
                    st = {}
                    emit_proj_mm(nt_i, st)
                    emit_proj_fin(nt_i, st)
                return [lambda nt_i=nt_i: one(nt_i)
                        for nt_i in range(4 * cc, 4 * cc + 4)]

            # ---------------- emission schedule ------------------------------
            # q01/k01 with kb interleaved for DMA slack, then v01; heads 0/1
            # of chunk 0 can then start, with the q23/k23/v23 projections and
            # later the cp1 projections riding the filler queue.
            emit_qkv_pair_interleaved(0, 0, 2)
            emit_qkv_jt(0, 4)
            fillers += make_qkv_fillers(0, jts=(1, 3, 5))
            emit_attn_pair(0, 0)            # 4 fill slots
            while fillers:                  # finish heads 2/3 q/k/v
                fill_one()
            fillers += make_qkv_fillers(1)  # n in [1024, 2048) rides along
            flush_norms()
            emit_attn_pair(0, 1)            # 4 fill slots
            for hp in range(2):
                flush_norms()
                emit_attn_pair(1, hp)       # 16 fill slots
            while fillers:                  # make sure cp1 q/k/v is emitted
                fill_one()
            fillers += make_proj_fillers(0)
            fillers += make_proj_fillers(1)
            fillers += make_proj_fillers(2)
            for hp in range(2):
                flush_norms()
                emit_attn_pair(2, hp, fill_every=4)   # 24 slots, 6 pops
            for hp in range(2):
                flush_norms()
                emit_attn_pair(3, hp, fill_every=4)   # 32 slots, 8 pops
            flush_norms()
            while fillers:
                fill_one()
            for nt_i in range(12, 16):
                st = {}
                emit_proj_mm(nt_i, st)
                emit_proj_fin(nt_i, st)
            assert out_n[0] == NT, f"emitted {out_n[0]} proj tiles"

    nc.compile()
    return nc


def _get_program():
    if "nc" not in _CACHE:
        _CACHE["nc"] = _build_program()
    return _CACHE["nc"]


def _rot_lhsT():
    """lhsT for rot_half: out = lhsT.T @ tT = R @ tT, interleaved pairs."""
    R64 = np.zeros((64, 64), np.float32)
    for i in range(32):
        R64[2 * i, 2 * i + 1] = -1.0
        R64[2 * i + 1, 2 * i] = 1.0
    R = np.zeros((128, 128), np.float32)
    R[0:64, 0:64] = R64
    R[64:128, 64:128] = R64
    return np.ascontiguousarray(R.T)


def make_in_maps(x, rotary_pos_emb, w_qkv, w_out, b_out):
    x = np.asarray(x, np.float32)
    rotary_pos_emb = np.asarray(rotary_pos_emb, np.float32)
    w_qkv = np.asarray(w_qkv, np.float32)
    w_out = np.asarray(w_out, np.float32)

    import ml_dtypes
    bf16 = ml_dtypes.bfloat16
    identb = np.eye(128).astype(bf16)
    rmatT = _rot_lhsT()
    freqsT = np.ascontiguousarray(rotary_pos_emb.T)

    in_maps = []
    for c in range(NCORES):
        b = c // 4
        heads = [4 * (c % 4) + i for i in range(HPC)]
        # w_qkv column shard in j-tile order: q01,q23,k01,k23,v01,v23
        cols = []
        for t in range(3):            # q, k, v
            for h in heads:
                cols.append(w_qkv[:, t * H * DH + h * DH: t * H * DH + (h + 1) * DH])
        w_s = np.ascontiguousarray(np.concatenate(cols, axis=1))
        w_o = np.ascontiguousarray(
            np.concatenate([w_out[h * DH:(h + 1) * DH, :] for h in heads], axis=0))
        in_maps.append({
            "xT": np.ascontiguousarray(x[b].T).astype(bf16),
            "wqkv": w_s.astype(bf16),
            "wout": w_o.astype(bf16),
            "freqsT": freqsT,
            "rmatD": rmatT.astype(bf16),
            "identB": identb,
        })
    return in_maps


def kernel(x, rotary_pos_emb, w_qkv, w_out, b_out):
    from concourse.bass_utils import run_bass_kernel_spmd

    nc = _get_program()
    in_maps = make_in_maps(x, rotary_pos_emb, w_qkv, w_out, b_out)
    res = run_bass_kernel_spmd(nc, in_maps, list(range(NCORES))).results

    b_out = np.asarray(b_out, np.float32)
    out = np.zeros((B, N, DIM), np.float32)
    for c in range(NCORES):
        out[c // 4] += np.asarray(res[c]["out"], np.float32)
    out += b_out[None, None, :]
    return out


# revision 60
# speedup vs baseline: 1.2971x; 1.0277x over previous
"""Trainium2 Bass kernel for nn_Attention_43946105373274.

Causal multi-head attention with rotary embeddings applied to q, k and v.
B=2, N=2048, DIM=1024, H=16, DH=64, f32.

Sharding: 8 cores = (2 batches) x (4 head-groups of 4 heads).
Each core computes the qkv projection for its heads (w_qkv column-shard),
full causal attention for its heads, and a partial output projection
(w_out row-shard).  The host sums the 4 partials per batch and adds the
bias (the "all-reduce" of the output projection) — full inputs in, full
output out.

Per-core pipeline:
  0. x^T and freqs^T are transposed on the host and w_qkv is packed
     per jt-tile (layout prep only), so all device DMAs are wide
     contiguous rows.  The 16 DMA engines drain everything as one
     global FIFO (~350GB/s), so transfers are ordered by criticality —
     first jt's weights (256KB), first x^T pairs, freqs — letting the
     first matmul start ~15us in (after a fixed ~9us DGE warmup).
     cos/sin are computed on device from freqs^T; the round-to-nearest
     range-reduction steps ride the scalar engine's Copy scale+bias
     ports and the pi/2 cosine shift rides the Sin bias port.  No PE
     transposes and no fp32 PE instructions anywhere (fp32 is 4x
     slower and the Sin spline is only accurate in [-pi, pi]).
  1. QKV projection = bf16 matmuls accumulating in f32 PSUM, producing
     q/k/v in transposed [channels, n] layout (two heads per tile).
     Rotary: rotate_half via one bf16 R-matrix matmul; cos/sin combines
     on DVE in bf16.
  2. Attention is chunk-major with two heads interleaved, and a filler
     queue of independent tensor-engine work (the second half of the
     QKV projection, then output-projection tiles of the previous
     chunk) is popped between attention groups.  The PE therefore never
     idles while the scalar engine computes exp — idle gaps both waste
     cycles and demote the PE clock (2.4 -> 1.2 GHz) for multi-us
     epochs.  S^T = K-block^T q (f32 PSUM), exp on the scalar engine
     (DH^-0.5 folded into the exp scale; max-subtraction is
     mathematically redundant here), causal mask via gpsimd
     affine_select, AV accumulates with a ones-column appended to V so
     softmax row-sums fall out of the same matmul.  Normalization is
     split: the row-sum copy issues right after the last AV, the bf16
     ones-broadcast matmul + reciprocal + scale are deferred one head
     pair to stay off the critical path.
  3. Output-projection tiles ride the filler queue one chunk behind
     attention, so projection matmuls and the f16 output DMAs overlap
     the attention phase instead of serializing after it.
"""

import sys
import numpy as np

if "/opt/trn_rl_repo" not in sys.path:
    sys.path.insert(0, "/opt/trn_rl_repo")

B, N, DIM, H, DH = 2, 2048, 1024, 16, 64
HPC = 4                     # heads per core
NCORES = 8
SCALE = DH ** -0.5
NT = N // 128               # 16 row tiles
KB = DIM // 128             # 8 contraction blocks
CW = 512                    # i-chunk width
NCH = N // CW               # 4 chunks

_CACHE = {}


def _build_program():
    import concourse.bass as bass  # noqa: F401
    import concourse.mybir as mybir
    import concourse.tile as tile
    from concourse import bacc

    F32 = mybir.dt.float32
    F16 = mybir.dt.float16
    BF16 = mybir.dt.bfloat16
    AF = mybir.ActivationFunctionType
    OP = mybir.AluOpType

    nc = bacc.Bacc("TRN2", target_bir_lowering=False, debug=False,
                   num_devices=NCORES)

    xT = nc.dram_tensor("xT", [DIM, N], BF16, kind="ExternalInput")
    wqkv = nc.dram_tensor("wqkv", [6 * 128, DIM], BF16, kind="ExternalInput")
    wout = nc.dram_tensor("wout", [HPC * DH, DIM], BF16, kind="ExternalInput")
    freqsT = nc.dram_tensor("freqsT", [DH, N], F32, kind="ExternalInput")
    rmatD = nc.dram_tensor("rmatD", [128, 128], BF16, kind="ExternalInput")
    identB = nc.dram_tensor("identB", [128, 128], BF16, kind="ExternalInput")
    outD = nc.dram_tensor("out", [N, DIM], F16, kind="ExternalOutput")

    MAGIC = 12582912.0          # 1.5 * 2**23: float32 round-to-nearest trick
    TWO_PI = float(2 * np.pi)
    INV_2PI = float(1.0 / TWO_PI)
    HALF_PI = float(np.pi / 2)

    with tile.TileContext(nc) as tc:
        with tc.tile_pool(name="pc", bufs=1) as pc, \
             tc.tile_pool(name="pw", bufs=1) as pw, \
             tc.tile_pool(name="px", bufs=8) as px, \
             tc.tile_pool(name="pqk", bufs=4) as pqk, \
             tc.tile_pool(name="pv", bufs=4) as pv, \
             tc.tile_pool(name="pst", bufs=3) as pst, \
             tc.tile_pool(name="ppt", bufs=6) as ppt, \
             tc.tile_pool(name="poT", bufs=2) as poT, \
             tc.tile_pool(name="pnm", bufs=2) as pnm, \
             tc.tile_pool(name="pout", bufs=3) as pout, \
             tc.tile_pool(name="ptr", bufs=1) as ptr, \
             tc.tile_pool(name="psA", bufs=2, space="PSUM") as psA, \
             tc.tile_pool(name="psB", bufs=3, space="PSUM") as psB:

            # ---------------- phase 0: DMAs ----------------------------------
            # The 16 DMA engines drain transfers in a single global FIFO, so
            # queue-splitting buys nothing — order one queue by criticality:
            # constants, w_qkv, x^T cp0, freqs, x^T cp1, w_out.
            rmat = pc.tile([128, 128], BF16, tag="rmat")
            nc.sync.dma_start(rmat[:], rmatD[:])
            identb = pc.tile([128, 128], BF16, tag="identb")
            nc.sync.dma_start(identb[:], identB[:])

            # w_qkv arrives host-packed per jt-tile ([6*128, 1024], each jt
            # one contiguous 256KB transfer) so the first projection can
            # start after 0.25MB of weights instead of 1.5MB.
            wall = pw.tile([128, 6, KB, 128], BF16, tag="w")

            def dma_w(jt):
                nc.sync.dma_start(
                    wall[:, jt, :, :].rearrange("p kb j -> p (kb j)"),
                    wqkv[jt * 128:(jt + 1) * 128, :])

            def wview(jt, kb):
                return wall[:, jt, kb, :]

            xc = [px.tile([128, 2, 1024], BF16, tag="xT", name=f"xc_{cp}_{pr}")
                  for cp in range(2) for pr in range(4)]

            def xview(cp, kb):
                return xc[cp * 4 + kb // 2][:, kb % 2, :]

            def dma_xc(cp, prs):
                for pr in prs:          # pairs of kb
                    nc.sync.dma_start(
                        xc[cp * 4 + pr][:],
                        xT[pr * 256:(pr + 1) * 256,
                           cp * 1024:(cp + 1) * 1024].rearrange(
                               "(kb p) n -> p kb n", p=128))

            dma_w(0)
            dma_w(2)
            dma_xc(0, range(2))
            dma_w(4)

            sarg = ptr.tile([128, N], F32, tag="sarg")
            for half in range(2):
                nc.sync.dma_start(sarg[half * 64:(half + 1) * 64, :], freqsT[:])

            dma_xc(0, range(2, 4))
            for jt in (1, 3, 5):
                dma_w(jt)
            dma_xc(1, range(4))

            wot = pw.tile([128, 2, DIM], BF16, tag="wo")
            nc.sync.dma_start(
                wot[:], wout[:].rearrange("(cb p) d -> p cb d", p=128))
            wo_sb = [wot[:, cb, :] for cb in range(2)]

            ones_f = pc.tile([128, 128], F32, tag="ones_f")
            nc.vector.memset(ones_f[:], 1.0)
            ones_b = pc.tile([1, 128], BF16, tag="ones_b")
            nc.vector.tensor_copy(ones_b[:], ones_f[0:1, :])
            hpi_t = pc.tile([128, 1], F32, tag="hpi_t")
            nc.vector.memset(hpi_t[:], HALF_PI)
            zero_t = pc.tile([128, 1], F32, tag="zero_t")
            nc.vector.memset(zero_t[:], 0.0)

            # cos/sin in bf16, [128, N] (both 64-row halves identical).
            # Range-reduce to [-pi, pi] for the Sin spline; the
            # round-to-nearest (magic-constant) steps ride the scalar
            # engine's scale+bias ports so only one DVE op remains.
            # sin(y) = Sin(y - 2pi*round(y/2pi))
            # cos(y) = Sin((y - 2pi*round(y/2pi + 1/4)) + pi/2)
            cosT2 = pc.tile([128, N], BF16, tag="cosT2")
            sinT2 = pc.tile([128, N], BF16, tag="sinT2")
            for cc in range(NCH):
                sl = slice(cc * CW, (cc + 1) * CW)
                for dst, kofs, bias in ((sinT2, 0.0, zero_t[:]),
                                        (cosT2, 0.25, hpi_t[:])):
                    k = pst.tile([128, CW], F32, tag="trk", bufs=2)
                    nc.scalar.activation(k[:], sarg[:, sl], AF.Copy,
                                         scale=INV_2PI, bias=MAGIC + kofs)
                    nc.scalar.activation(k[:], k[:], AF.Copy, bias=-MAGIC)
                    xr = pst.tile([128, CW], F32, tag="trx", bufs=2)
                    nc.vector.scalar_tensor_tensor(xr[:], k[:], -TWO_PI,
                                                   sarg[:, sl],
                                                   op0=OP.mult, op1=OP.add)
                    nc.scalar.activation(dst[:, sl], xr[:], AF.Sin, bias=bias)

            # persistent tensors
            qT = [pqk.tile([128, N], BF16, tag="qk", name=f"qT{i}") for i in range(2)]
            kT = [pqk.tile([128, N], BF16, tag="qk", name=f"kT{i}") for i in range(2)]
            # V tiles: [128, 65] per (head, row-tile); col 64 = ones
            vt = [pv.tile([128, NT * (DH + 1)], BF16, tag="v", name=f"vt{h}", bufs=4)
                  for h in range(HPC)]
            for h in range(HPC):
                vv = vt[h][:].rearrange("p (t c) -> p t c", c=DH + 1)
                nc.vector.tensor_copy(vv[:, :, DH:DH + 1],
                                      ones_f[:, 0:NT].unsqueeze(2))
            oT = [poT.tile([128, N], BF16, tag="oT", name=f"oT{i}") for i in range(2)]

            # ---------------- phase 1: qkv projection + rotary ---------------
            def emit_rotary(cp, jt, half, ph, pstag):
                c = cp * 2 + half
                csl = cosT2[:, c * CW:(c + 1) * CW]
                ssl = sinT2[:, c * CW:(c + 1) * CW]
                psbufs = 1 if pstag == "qp" else None
                copy_eng = nc.vector if pstag == "qp" else nc.scalar
                t_sb = pst.tile([128, CW], BF16, tag="t_sb", bufs=6)
                if copy_eng is nc.scalar:
                    nc.scalar.activation(t_sb[:], ph, AF.Copy)
                else:
                    nc.vector.tensor_copy(t_sb[:], ph)
                rps = psB.tile([128, CW], F32, tag=pstag, bufs=psbufs,
                               name=f"rps_{cp}_{jt}_{half}")
                nc.tensor.matmul(rps[:], rmat[:], t_sb[:], start=True, stop=True)
                tmp = pst.tile([128, CW], BF16, tag="tmp", bufs=3)
                nc.vector.tensor_mul(tmp[:], t_sb[:], csl)
                rs = pst.tile([128, CW], BF16, tag="rs", bufs=3)
                nc.vector.tensor_mul(rs[:], rps[:], ssl)
                if jt < 4:  # q or k -> straight into qT/kT
                    dst = qT[jt] if jt < 2 else kT[jt - 2]
                    nc.vector.tensor_add(dst[:, c * CW:(c + 1) * CW],
                                         tmp[:], rs[:])
                else:       # v -> rotate then transpose into V tiles
                    v_sb = pst.tile([128, CW], BF16, tag="v_sb", bufs=2)
                    nc.vector.tensor_add(v_sb[:], tmp[:], rs[:])
                    pair = jt - 4
                    vps = psB.tile([128, CW], BF16, tag=pstag, bufs=psbufs,
                                   name=f"vps_{cp}_{pair}_{half}")
                    for rt in range(4):
                        nc.tensor.transpose(
                            vps[:, rt * 128:(rt + 1) * 128],
                            v_sb[:, rt * 128:(rt + 1) * 128],
                            identb[:])
                    vpsv = vps[:].rearrange("p (t hh d) -> p t hh d",
                                            t=4, hh=2)
                    for hh in range(2):
                        h = pair * 2 + hh
                        dstv = vt[h][:].rearrange(
                            "p (t c) -> p t c", c=DH + 1)[
                            :, c * 4:(c + 1) * 4, 0:DH]
                        nc.vector.tensor_copy(dstv, vpsv[:, :, hh, :])

            def qkv_mm(cp, jt, qps, kbr):
                for kb in kbr:
                    for mh in range(2):
                        nc.tensor.matmul(
                            qps[:, mh * 512:(mh + 1) * 512],
                            wview(jt, kb),
                            xview(cp, kb)[:, mh * 512:(mh + 1) * 512],
                            start=(kb == 0), stop=(kb == KB - 1))

            def emit_qkv_jt(cp, jt):
                qps = psA.tile([128, 1024], F32, tag="psA",
                               name=f"qps_{cp}_{jt}")
                qkv_mm(cp, jt, qps, range(KB))
                for half in range(2):
                    emit_rotary(cp, jt, half,
                                qps[:, half * 512:(half + 1) * 512], "ps512")

            def emit_qkv_pair_interleaved(cp, jta, jtb):
                # kb halves interleaved across two jts so the tensor engine
                # has runway while the later x^T pairs are still in flight
                qa = psA.tile([128, 1024], F32, tag="psA", name=f"qps_{cp}_{jta}")
                qkv_mm(cp, jta, qa, range(0, 4))
                qb = psA.tile([128, 1024], F32, tag="psA", name=f"qps_{cp}_{jtb}")
                qkv_mm(cp, jtb, qb, range(0, 4))
                qkv_mm(cp, jta, qa, range(4, KB))
                for half in range(2):
                    emit_rotary(cp, jta, half,
                                qa[:, half * 512:(half + 1) * 512], "ps512")
                qkv_mm(cp, jtb, qb, range(4, KB))
                for half in range(2):
                    emit_rotary(cp, jtb, half,
                                qb[:, half * 512:(half + 1) * 512], "ps512")

            def make_qkv_fillers(cp, jts=(0, 2, 4, 1, 3, 5)):
                # Each (jt, half) is two pops: the 512-wide projection into a
                # dedicated 2KB PSUM slot (tag "qp", never contended by the
                # attention accumulators), then the rotary combine.
                out = []
                for jt in jts:
                    for half in range(2):
                        st = {}
                        def fa(jt=jt, half=half, st=st):
                            st['qp'] = psB.tile([128, CW], F32, tag="qp",
                                                bufs=1,
                                                name=f"qp_{cp}_{jt}_{half}")
                            for kb in range(KB):
                                nc.tensor.matmul(
                                    st['qp'][:],
                                    wview(jt, kb),
                                    xview(cp, kb)[:, half * 512:(half + 1) * 512],
                                    start=(kb == 0), stop=(kb == KB - 1))
                        def fb(jt=jt, half=half, st=st):
                            emit_rotary(cp, jt, half, st['qp'][:], "qp")
                        out += [fa, fb]
                return out

            # ---------------- phase 2: attention + projection ----------------
            fillers = []

            def fill_one():
                if fillers:
                    fillers.pop(0)()

            pending_norm = []

            def emit_norm2(h, av_t, s_rb, cc):
                pair, hh = h // 2, h % 2
                rbp = psB.tile([64, CW], F32, tag="ps512",
                               name=f"rbp_{h}_{cc}")
                nc.tensor.matmul(rbp[:], ones_b[0:1, 0:64], s_rb[:],
                                 start=True, stop=True)
                rb = pnm.tile([64, CW], F32, tag="rb", bufs=2,
                              name=f"rb_{h}_{cc}")
                nc.vector.reciprocal_approx_fast(rb[:], rbp[:])
                osl = oT[pair][hh * 64:(hh + 1) * 64, cc * CW:(cc + 1) * CW]
                nc.vector.tensor_mul(osl, av_t[0:DH, :], rb[:])

            def flush_norms():
                while pending_norm:
                    emit_norm2(*pending_norm.pop(0))

            def emit_attn_pair(c, hp, fill_every=1):
                it = [0]
                nj = 4 * c + 4          # j-blocks needed (causal)
                avs = {}
                for hh in range(2):
                    h = 2 * hp + hh
                    avs[h] = psB.tile([DH + 1, CW], F32, tag="ps512",
                                      name=f"av_{h}_{c}")
                # AV matmuls run two iterations behind their exp so the
                # in-order tensor engine never waits on the scalar engine.
                pend_av = []

                def pop_av():
                    pend_av.pop(0)()

                for grp in range(nj // 2):
                    j0 = grp * 2
                    diag = j0 + 1 >= 4 * c   # group touches the diagonal
                    for hh in range(2):
                        h = 2 * hp + hh
                        pair = h // 2
                        qh = qT[pair][hh * 64:(hh + 1) * 64, :]
                        kh = kT[pair][hh * 64:(hh + 1) * 64, :]
                        sps = psA.tile([128, 1024], F32, tag="psA",
                                       name=f"sps_{h}_{c}_{grp}")
                        pt = ppt.tile([128, 1024], BF16, tag="pt", bufs=6)
                        if not diag:
                            for g in range(2):
                                j = j0 + g
                                nc.tensor.matmul(
                                    sps[:, g * 512:(g + 1) * 512],
                                    kh[:, j * 128:(j + 1) * 128],
                                    qh[:, c * CW:(c + 1) * CW],
                                    start=True, stop=True)
                            nc.scalar.activation(pt[:], sps[:], AF.Exp,
                                                 scale=SCALE)

                            def do_av(h=h, j0=j0, pt=pt):
                                for g in range(2):
                                    j = j0 + g
                                    nc.tensor.matmul(
                                        avs[h][:],
                                        vt[h][:, j * (DH + 1):(j + 1) * (DH + 1)],
                                        pt[:, g * 512:(g + 1) * 512],
                                        start=(j == 0), stop=(j == nj - 1))
                        else:
                            # diagonal blocks: columns i < goff are fully
                            # masked — skip their S/exp/AV entirely; only a
                            # fixed 128-wide triangle needs affine_select.
                            for g in range(2):
                                j = j0 + g
                                goff = (j - 4 * c) * 128
                                sl = slice(g * 512 + goff, (g + 1) * 512)
                                nc.tensor.matmul(
                                    sps[:, sl],
                                    kh[:, j * 128:(j + 1) * 128],
                                    qh[:, c * CW + goff:(c + 1) * CW],
                                    start=True, stop=True)
                                nc.scalar.activation(pt[:, sl], sps[:, sl],
                                                     AF.Exp, scale=SCALE)
                                tri = pt[:, g * 512 + goff:
                                         g * 512 + goff + 128]
                                nc.gpsimd.affine_select(
                                    out=tri, in_=tri,
                                    compare_op=OP.is_ge, fill=0.0,
                                    base=0, pattern=[[1, 128]],
                                    channel_multiplier=-1)

                            def do_av(h=h, j0=j0, pt=pt, c=c):
                                for g in range(2):
                                    j = j0 + g
                                    goff = (j - 4 * c) * 128
                                    nc.tensor.matmul(
                                        avs[h][:, goff:],
                                        vt[h][:, j * (DH + 1):(j + 1) * (DH + 1)],
                                        pt[:, g * 512 + goff:(g + 1) * 512],
                                        start=(j == 0), stop=(j == nj - 1),
                                        skip_group_check=True)
                        pend_av.append(do_av)
                        if len(pend_av) > 2:
                            pop_av()
                        if it[0] % fill_every == 0:
                            fill_one()
                        it[0] += 1
                while pend_av:
                    pop_av()
                for hh in range(2):
                    h = 2 * hp + hh
                    s_rb = pnm.tile([1, CW], BF16, tag="s_rb", bufs=2,
                                    name=f"s_rb_{h}_{c}")
                    nc.vector.tensor_copy(s_rb[:], avs[h][DH:DH + 1, :])
                    pending_norm.append((h, avs[h], s_rb, c))

            out_n = [0]

            def emit_proj_mm(nt_i, st):
                st['prj'] = psA.tile([128, DIM], F32, tag="psA",
                                     name=f"prj_{nt_i}")
                for mh in range(2):
                    for cb in range(2):
                        nc.tensor.matmul(
                            st['prj'][:, mh * 512:(mh + 1) * 512],
                            oT[cb][:, nt_i * 128:(nt_i + 1) * 128],
                            wo_sb[cb][:, mh * 512:(mh + 1) * 512],
                            start=(cb == 0), stop=(cb == 1))

            def emit_proj_fin(nt_i, st):
                ot = pout.tile([128, DIM], F16, tag="osb", bufs=3,
                               name=f"ot_{nt_i}")
                nc.vector.tensor_copy(ot[:], st['prj'][:])
                eng = nc.scalar if nt_i % 2 else nc.sync
                eng.dma_start(outD[nt_i * 128:(nt_i + 1) * 128, :], ot[:])
                out_n[0] += 1

            def make_proj_fillers(cc):
                def one(nt_i):
                    st = {}
                    emit_proj_mm(nt_i, st)
                    emit_proj_fin(nt_i, st)
                return [lambda nt_i=nt_i: one(nt_i)
                        for nt_i in range(4 * cc, 4 * cc + 4)]

            # ---------------- emission schedule ------------------------------
            # q01/k01 with kb interleaved for DMA slack, then v01; heads 0/1
            # of chunk 0 can then start, with the q23/k23/v23 projections and
            # later the cp1 projections riding the filler queue.
            emit_qkv_pair_interleaved(0, 0, 2)
            emit_qkv_jt(0, 4)
            fillers += make_qkv_fillers(0, jts=(1, 3, 5))
            emit_attn_pair(0, 0)            # 4 fill slots
            while fillers:                  # finish heads 2/3 q/k/v
                fill_one()
            fillers += make_qkv_fillers(1)  # n in [1024, 2048) rides along
            flush_norms()
            emit_attn_pair(0, 1)            # 4 fill slots
            for hp in range(2):
                flush_norms()
                emit_attn_pair(1, hp)       # 16 fill slots
            while fillers:                  # make sure cp1 q/k/v is emitted
                fill_one()
            fillers += make_proj_fillers(0)
            fillers += make_proj_fillers(1)
            fillers += make_proj_fillers(2)
            for hp in range(2):
                flush_norms()
                emit_attn_pair(2, hp, fill_every=4)   # 24 slots, 6 pops
            for hp in range(2):
                flush_norms()
                emit_attn_pair(3, hp, fill_every=4)   # 32 slots, 8 pops
            flush_norms()
            while fillers:
                fill_one()
            for nt_i in range(12, 16):
                st = {}
                emit_proj_mm(nt_i, st)
                emit_proj_fin(nt_i, st)
            assert out_n[0] == NT, f"emitted {out_n[0]} proj tiles"

    nc.compile()
    return nc


def _get_program():
    if "nc" not in _CACHE:
        _CACHE["nc"] = _build_program()
    return _CACHE["nc"]


def _rot_lhsT():
    """lhsT for rot_half: out = lhsT.T @ tT = R @ tT, interleaved pairs."""
    R64 = np.zeros((64, 64), np.float32)
    for i in range(32):
        R64[2 * i, 2 * i + 1] = -1.0
        R64[2 * i + 1, 2 * i] = 1.0
    R = np.zeros((128, 128), np.float32)
    R[0:64, 0:64] = R64
    R[64:128, 64:128] = R64
    return np.ascontiguousarray(R.T)


def make_in_maps(x, rotary_pos_emb, w_qkv, w_out, b_out):
    x = np.asarray(x, np.float32)
    rotary_pos_emb = np.asarray(rotary_pos_emb, np.float32)
    w_qkv = np.asarray(w_qkv, np.float32)
    w_out = np.asarray(w_out, np.float32)

    import ml_dtypes
    bf16 = ml_dtypes.bfloat16
    identb = np.eye(128).astype(bf16)
    rmatT = _rot_lhsT()
    freqsT = np.ascontiguousarray(rotary_pos_emb.T)

    in_maps = []
    for c in range(NCORES):
        b = c // 4
        heads = [4 * (c % 4) + i for i in range(HPC)]
        # w_qkv column shard in j-tile order: q01,q23,k01,k23,v01,v23
        cols = []
        for t in range(3):            # q, k, v
            for h in heads:
                cols.append(w_qkv[:, t * H * DH + h * DH: t * H * DH + (h + 1) * DH])
        w_s = np.ascontiguousarray(np.concatenate(cols, axis=1))
        w_o = np.ascontiguousarray(
            np.concatenate([w_out[h * DH:(h + 1) * DH, :] for h in heads], axis=0))
        in_maps.append({
            "xT": np.ascontiguousarray(x[b].T).astype(bf16),
            "wqkv": np.ascontiguousarray(
                w_s.reshape(8, 128, 6, 128).transpose(2, 1, 0, 3)
                .reshape(6 * 128, 1024)).astype(bf16),
            "wout": w_o.astype(bf16),
            "freqsT": freqsT,
            "rmatD": rmatT.astype(bf16),
            "identB": identb,
        })
    return in_maps


def kernel(x, rotary_pos_emb, w_qkv, w_out, b_out):
    from concourse.bass_utils import run_bass_kernel_spmd

    nc = _get_program()
    in_maps = make_in_maps(x, rotary_pos_emb, w_qkv, w_out, b_out)
    res = run_bass_kernel_spmd(nc, in_maps, list(range(NCORES))).results

    b_out = np.asarray(b_out, np.float32)
    out = np.zeros((B, N, DIM), np.float32)
    for c in range(NCORES):
        out[c // 4] += np.asarray(res[c]["out"], np.float32)
    out += b_out[None, None, :]
    return out
